# revision 9
# baseline (speedup 1.0000x reference)
"""AgglutinativeAttention Trainium2 kernel.

Full inputs in, full output out. Sharding: 8 cores = (batch b in 0..3) x
(head-group g in 0..1). Each core computes, for its batch b and its 8 heads:
  qT/kT = (x @ W{q,k}[:, gF:(g+1)F])^T   [512 feat, 1024 tok]  (q pre-scaled)
  v     =  x @ Wv[:, gF:(g+1)F]          [1024 tok, 512 feat] (+ones col/head)
  per head: sT = kT_h^T-style scores transposed [j, i] via PE,
  morpho bias injected via identity-matmul (verb one-hot) + per-partition
  activation bias (col bias), pT = exp(sT + bias), oT = v_aug^T @ pT with a
  ones row giving the softmax denominator, divide, then partial
  z = o @ Wo[gF:(g+1)F, :].  Host sums the two per-batch partials + bo.
"""

import numpy as np
from contextlib import ExitStack

import concourse.bass as bass
import concourse.mybir as mybir
import concourse.tile as tile
from concourse import bacc
from concourse.bass_utils import run_bass_kernel_spmd
from concourse.masks import make_identity

B, S, H = 4, 1024, 1024
NH, HD = 16, 64
G = 2                 # head groups (tensor-parallel factor per batch)
F = H // G            # 512 features per core
HPC = NH // G         # 8 heads per core
SCALE = 1.0 / np.sqrt(HD)
VERB_BIAS, ROOT_BIAS, SUFFIX_BIAS = 2.0, 1.5, 1.2
BIG = np.float32(1e9)

f32 = mybir.dt.float32
f32r = mybir.dt.float32r
i32 = mybir.dt.int32

P = 128
KC = H // P           # 8 contraction chunks for projections
TC = S // P           # 8 token chunks of 128
IC = S // 512         # 2 chunks of 512 (matmul free dim)
FC = F // P           # 4 feature chunks per core

_COMPILED = None


def _build():
    nc = bacc.Bacc("TRN2", target_bir_lowering=False, debug=False, num_devices=8)

    x_d = nc.dram_tensor("x", [S, H], f32, kind="ExternalInput").ap()
    wq_d = nc.dram_tensor("wq", [H, F], f32, kind="ExternalInput").ap()
    wk_d = nc.dram_tensor("wk", [H, F], f32, kind="ExternalInput").ap()
    wv_d = nc.dram_tensor("wv", [H, F], f32, kind="ExternalInput").ap()
    wo_d = nc.dram_tensor("wo", [F, H], f32, kind="ExternalInput").ap()
    bqs_d = nc.dram_tensor("bqs", [F], f32, kind="ExternalInput").ap()
    bk_d = nc.dram_tensor("bk", [F], f32, kind="ExternalInput").ap()
    bv_d = nc.dram_tensor("bv", [F], f32, kind="ExternalInput").ap()
    nearf_d = nc.dram_tensor("nearf", [S], f32, kind="ExternalInput").ap()
    cb_d = nc.dram_tensor("cb", [S], f32, kind="ExternalInput").ap()
    z_d = nc.dram_tensor("z", [S, H], f32, kind="ExternalOutput").ap()

    with tile.TileContext(nc) as tc, ExitStack() as ctx:
        const = ctx.enter_context(tc.tile_pool(name="const", bufs=1))
        big = ctx.enter_context(tc.tile_pool(name="big", bufs=1))
        ppool = ctx.enter_context(tc.tile_pool(name="ppool", bufs=4))
        rlpool = ctx.enter_context(tc.tile_pool(name="rlpool", bufs=2))
        zpool = ctx.enter_context(tc.tile_pool(name="zpool", bufs=3))
        ps_t = ctx.enter_context(tc.tile_pool(name="ps_t", bufs=2, space="PSUM"))
        ps_q = ctx.enter_context(tc.tile_pool(name="ps_q", bufs=2, space="PSUM"))
        ps_s = ctx.enter_context(tc.tile_pool(name="ps_s", bufs=2, space="PSUM"))
        ps_o = ctx.enter_context(tc.tile_pool(name="ps_o", bufs=2, space="PSUM"))

        # ---- constants ----
        ident = const.tile([P, P], f32, tag="ident")
        make_identity(nc, ident)
        ident_r = const.tile([P, P], f32r, tag="ident_r")
        nc.vector.tensor_copy(ident_r[:], ident[:])
        iota_i = const.tile([P, KC], i32, tag="iota_i")
        nc.gpsimd.iota(iota_i[:], pattern=[[P, KC]], base=0, channel_multiplier=1)
        iota_f = const.tile([P, KC], f32, tag="iota_f")
        nc.vector.tensor_copy(iota_f[:], iota_i[:])

        cb_sb = const.tile([P, TC], f32, tag="cb_sb")
        nc.sync.dma_start(cb_sb[:], cb_d.rearrange("(jc p) -> p jc", p=P))
        near_row = const.tile([1, S], f32, tag="near_row")
        nc.sync.dma_start(near_row[:], nearf_d[None, :])
        near_bc = const.tile([P, S], f32, tag="near_bc")
        nc.gpsimd.partition_broadcast(near_bc[:], near_row[:])

        bq_sb = const.tile([P, FC], f32, tag="bq_sb")
        nc.sync.dma_start(bq_sb[:], bqs_d.rearrange("(fc p) -> p fc", p=P))
        bk_sb = const.tile([P, FC], f32, tag="bk_sb")
        nc.sync.dma_start(bk_sb[:], bk_d.rearrange("(fc p) -> p fc", p=P))
        bv_row = const.tile([1, F], f32, tag="bv_row")
        nc.sync.dma_start(bv_row[:], bv_d[None, :])
        bv_bc = const.tile([P, F], f32, tag="bv_bc")
        nc.gpsimd.partition_broadcast(bv_bc[:], bv_row[:])

        qT = big.tile([P, FC, S], f32r, tag="qT")
        kT = big.tile([P, FC, S], f32r, tag="kT")
        v_sb = big.tile([P, TC, HPC, 65], f32r, tag="v_sb")
        ones64 = const.tile([P, TC * HPC], f32, tag="ones64")
        nc.vector.memset(ones64[:], 1.0)
        nc.vector.tensor_copy(
            v_sb[:, :, :, 64:65],
            ones64.rearrange("p (a b one) -> p a b one", a=TC, b=HPC, one=1),
        )

        # ---- phase 1: transpose x, projections (weights + xT freed after) ----
        with tc.tile_pool(name="projpool", bufs=1) as projpool, \
             tc.tile_pool(name="xstage", bufs=2) as xstage:
            wq_sb = projpool.tile([P, KC, F], f32r, tag="wq_sb")
            nc.sync.dma_start(wq_sb[:], wq_d.bitcast(f32r).rearrange("(kc p) f -> p kc f", p=P))
            wk_sb = projpool.tile([P, KC, F], f32r, tag="wk_sb")
            nc.sync.dma_start(wk_sb[:], wk_d.bitcast(f32r).rearrange("(kc p) f -> p kc f", p=P))
            wv_sb = projpool.tile([P, KC, F], f32r, tag="wv_sb")
            nc.sync.dma_start(wv_sb[:], wv_d.bitcast(f32r).rearrange("(kc p) f -> p kc f", p=P))

            # transpose x -> xT [128, kc, tok]
            xT = projpool.tile([P, KC, S], f32r, tag="xT")
            for tci in range(TC):
                xt = xstage.tile([P, H], f32, tag="xt")
                nc.sync.dma_start(xt[:], x_d[tci * P : (tci + 1) * P, :])
                for hc in range(KC):
                    pst = ps_t.tile([P, P], f32, tag="pst")
                    nc.tensor.transpose(pst[:], xt[:, hc * P : (hc + 1) * P], ident[:])
                    nc.vector.tensor_copy(xT[:, hc, tci * P : (tci + 1) * P], pst[:])

            # q/k projections (transposed layout)
            for dst, wsb, bsb, scale in ((qT, wq_sb, bq_sb, SCALE), (kT, wk_sb, bk_sb, 1.0)):
                for fc in range(FC):
                    for ic in range(IC):
                        ps = ps_q.tile([P, 512], f32, tag="ps_proj")
                        for kc in range(KC):
                            nc.tensor.matmul(
                                ps[:],
                                wsb[:, kc, fc * P : (fc + 1) * P],
                                xT[:, kc, ic * 512 : (ic + 1) * 512],
                                start=(kc == 0), stop=(kc == KC - 1),
                            )
                        nc.scalar.activation(
                            dst[:, fc, ic * 512 : (ic + 1) * 512], ps[:],
                            mybir.ActivationFunctionType.Identity,
                            bias=bsb[:, fc : fc + 1], scale=scale,
                        )

            # v natural [tok, feat] + ones column per head -> [128, tc, h, 65]
            for tci in range(TC):
                ps = ps_q.tile([P, 512], f32, tag="ps_proj")
                for kc in range(KC):
                    nc.tensor.matmul(
                        ps[:],
                        xT[:, kc, tci * P : (tci + 1) * P],
                        wv_sb[:, kc, :],
                        start=(kc == 0), stop=(kc == KC - 1),
                    )
                nc.vector.tensor_tensor(
                    v_sb[:, tci, :, 0:64],
                    ps.rearrange("p (h d) -> p h d", d=64),
                    bv_bc.rearrange("p (h d) -> p h d", d=64),
                    mybir.AluOpType.add,
                )

        attn2 = ctx.enter_context(tc.tile_pool(name="attn2", bufs=1))
        # verb one-hot (transposed): oh2T[p, jc, i] = 2 * (jc*128+p == nearest[i])
        oh2T = attn2.tile([P, TC, S], f32r, tag="oh2T")
        for jc in range(TC):
            nc.vector.tensor_scalar(
                oh2T[:, jc, :], near_bc[:], iota_f[:, jc : jc + 1], 2.0,
                mybir.AluOpType.is_equal, mybir.AluOpType.mult,
            )

        # ---- attention ----
        oT = attn2.tile([64, HPC, S], f32r, tag="oT")
        for h in range(HPC):
            hb = (h % 2) * 64
            fc4 = h // 2
            for ic in range(IC):
                pso = ps_o.tile([65, 512], f32, tag="pso")
                for jc in range(TC):
                    pss = ps_s.tile([P, 512], f32, tag="pss")
                    nc.tensor.matmul(
                        pss[:],
                        kT[hb : hb + 64, fc4, jc * P : (jc + 1) * P],
                        qT[hb : hb + 64, fc4, ic * 512 : (ic + 1) * 512],
                        start=True, stop=False, skip_group_check=True,
                    )
                    nc.tensor.matmul(
                        pss[:],
                        ident_r[:],
                        oh2T[:, jc, ic * 512 : (ic + 1) * 512],
                        start=False, stop=True, skip_group_check=True,
                    )
                    pT = ppool.tile([P, 512], f32r, tag="pT")
                    nc.scalar.activation(
                        pT[:], pss[:], mybir.ActivationFunctionType.Exp,
                        bias=cb_sb[:, jc : jc + 1], scale=1.0,
                    )
                    nc.tensor.matmul(
                        pso[:],
                        v_sb[:, jc, h, 0:65],
                        pT[:],
                        start=(jc == 0), stop=(jc == TC - 1),
                    )
                rl = rlpool.tile([65, 512], f32, tag="rl")
                nc.vector.reciprocal(rl[64:65, :], pso[64:65, :])
                rlrow = rlpool.tile([1, 512], f32, tag="rlrow")
                nc.sync.dma_start(rlrow[:], rl[64:65, :])
                rlb = rlpool.tile([64, 512], f32, tag="rlb")
                nc.gpsimd.partition_broadcast(rlb[:], rlrow[:])
                nc.vector.tensor_tensor(
                    oT[:, h, ic * 512 : (ic + 1) * 512], pso[0:64, :], rlb[:],
                    mybir.AluOpType.mult,
                )

        # ---- output projection (partial) ----
        wo_sb = attn2.tile([64, HPC, H], f32r, tag="wo_sb")
        nc.sync.dma_start(wo_sb[:], wo_d.bitcast(f32r).rearrange("(h p) o -> p h o", p=64))
        for tci in range(TC):
            for oc in range(IC):
                psz = ps_q.tile([P, 512], f32, tag="ps_proj")
                for h in range(HPC):
                    nc.tensor.matmul(
                        psz[:],
                        oT[:, h, tci * P : (tci + 1) * P],
                        wo_sb[:, h, oc * 512 : (oc + 1) * 512],
                        start=(h == 0), stop=(h == HPC - 1),
                    )
                zt = zpool.tile([P, 512], f32, tag="zt")
                nc.scalar.copy(zt[:], psz[:])
                nc.sync.dma_start(z_d[tci * P : (tci + 1) * P, oc * 512 : (oc + 1) * 512], zt[:])

    nc.compile()
    return nc


def _get_compiled():
    global _COMPILED
    if _COMPILED is None:
        _COMPILED = _build()
    return _COMPILED


def _host_morpho(morpho_types):
    """nearest-verb index per (b, i) (-1 if batch has no verb) and col bias."""
    mt = np.asarray(morpho_types)
    pos = np.arange(S)
    dist = np.abs(pos[:, None] - pos[None, :]).astype(np.float32)
    nearest = np.empty((B, S), np.float32)
    for b in range(B):
        is_verb = mt[b] == 2
        if not is_verb.any():
            nearest[b] = -1.0
            continue
        dm = np.where(is_verb[None, :], dist, BIG)
        nearest[b] = np.argmin(dm, axis=-1).astype(np.float32)
    cb = (
        np.float32(ROOT_BIAS * 0.5) * (mt == 0)
        + np.float32(SUFFIX_BIAS * 0.3) * (mt == 1)
    ).astype(np.float32)
    return nearest, cb


def kernel(hidden_states, morpho_types, Wq, bq, Wk, bk, Wv, bv, Wo, bo):
    hidden_states = np.ascontiguousarray(np.asarray(hidden_states, np.float32))
    Wq = np.asarray(Wq, np.float32)
    Wk = np.asarray(Wk, np.float32)
    Wv = np.asarray(Wv, np.float32)
    Wo = np.asarray(Wo, np.float32)
    bq = np.asarray(bq, np.float32)
    bk = np.asarray(bk, np.float32)
    bv = np.asarray(bv, np.float32)
    bo = np.asarray(bo, np.float32)

    nearest, cb = _host_morpho(morpho_types)

    nc = _get_compiled()
    in_maps = []
    for c in range(8):
        b, g = c // G, c % G
        fs = slice(g * F, (g + 1) * F)
        in_maps.append({
            "x": hidden_states[b],
            "wq": np.ascontiguousarray(Wq[:, fs]),
            "wk": np.ascontiguousarray(Wk[:, fs]),
            "wv": np.ascontiguousarray(Wv[:, fs]),
            "wo": np.ascontiguousarray(Wo[fs, :]),
            "bqs": np.ascontiguousarray(bq[fs]) * np.float32(SCALE),
            "bk": np.ascontiguousarray(bk[fs]),
            "bv": np.ascontiguousarray(bv[fs]),
            "nearf": nearest[b],
            "cb": cb[b],
        })

    res = run_bass_kernel_spmd(nc, in_maps, core_ids=list(range(8)))
    out = np.empty((B, S, H), np.float32)
    for b in range(B):
        out[b] = res.results[2 * b]["z"] + res.results[2 * b + 1]["z"] + bo
    return out


# revision 21
# speedup vs baseline: 82.8075x; 82.8075x over previous
"""AgglutinativeAttention Trainium2 kernel.

Full inputs in, full output out. Sharding: 8 cores = (batch b in 0..3) x
(head-group g in 0..1). Each core computes, for its batch b and its 8 heads:
  qT/kT = (x @ W{q,k}[:, gF:(g+1)F])^T   [512 feat, 1024 tok]  (q pre-scaled)
  v     =  x @ Wv[:, gF:(g+1)F]          [1024 tok, 512 feat] (+ones col/head)
  per head: sT = kT_h^T-style scores transposed [j, i] via PE,
  morpho bias injected via identity-matmul (verb one-hot) + per-partition
  activation bias (col bias), pT = exp(sT + bias), oT = v_aug^T @ pT with a
  ones row giving the softmax denominator, divide, then partial
  z = o @ Wo[gF:(g+1)F, :].  Host sums the two per-batch partials + bo.
"""

import numpy as np
from contextlib import ExitStack

import concourse.bass as bass
import concourse.mybir as mybir
import concourse.tile as tile
from concourse import bacc
from concourse.bass_utils import run_bass_kernel_spmd
from concourse.masks import make_identity

B, S, H = 4, 1024, 1024
NH, HD = 16, 64
G = 2                 # head groups (tensor-parallel factor per batch)
F = H // G            # 512 features per core
HPC = NH // G         # 8 heads per core
SCALE = 1.0 / np.sqrt(HD)
VERB_BIAS, ROOT_BIAS, SUFFIX_BIAS = 2.0, 1.5, 1.2
BIG = np.float32(1e9)

f32 = mybir.dt.float32
f32r = mybir.dt.float32r
i32 = mybir.dt.int32

P = 128
KC = H // P           # 8 contraction chunks for projections
TC = S // P           # 8 token chunks of 128
IC = S // 512         # 2 chunks of 512 (matmul free dim)
FC = F // P           # 4 feature chunks per core

_COMPILED = None


def _build():
    nc = bacc.Bacc("TRN2", target_bir_lowering=False, debug=False, num_devices=8)

    x_d = nc.dram_tensor("x", [S, H], f32, kind="ExternalInput").ap()
    wq_d = nc.dram_tensor("wq", [H, F], f32, kind="ExternalInput").ap()
    wk_d = nc.dram_tensor("wk", [H, F], f32, kind="ExternalInput").ap()
    wv_d = nc.dram_tensor("wv", [H, F], f32, kind="ExternalInput").ap()
    wo_d = nc.dram_tensor("wo", [F, H], f32, kind="ExternalInput").ap()
    bqs_d = nc.dram_tensor("bqs", [F], f32, kind="ExternalInput").ap()
    bk_d = nc.dram_tensor("bk", [F], f32, kind="ExternalInput").ap()
    bv_d = nc.dram_tensor("bv", [F], f32, kind="ExternalInput").ap()
    nearf_d = nc.dram_tensor("nearf", [S], f32, kind="ExternalInput").ap()
    cb_d = nc.dram_tensor("cb", [S], f32, kind="ExternalInput").ap()
    z_d = nc.dram_tensor("z", [S, H], f32, kind="ExternalOutput").ap()

    bf16 = mybir.dt.bfloat16

    with tile.TileContext(nc) as tc, ExitStack() as ctx:
        const = ctx.enter_context(tc.tile_pool(name="const", bufs=1))
        big = ctx.enter_context(tc.tile_pool(name="big", bufs=1))
        ppool = ctx.enter_context(tc.tile_pool(name="ppool", bufs=6))
        rlpool = ctx.enter_context(tc.tile_pool(name="rlpool", bufs=4))
        zpool = ctx.enter_context(tc.tile_pool(name="zpool", bufs=3))
        ps_q = ctx.enter_context(tc.tile_pool(name="ps_q", bufs=2, space="PSUM"))
        ps_s = ctx.enter_context(tc.tile_pool(name="ps_s", bufs=2, space="PSUM"))
        ps_o = ctx.enter_context(tc.tile_pool(name="ps_o", bufs=4, space="PSUM"))

        # ---- constants ----
        ident = const.tile([P, P], f32, tag="ident")
        make_identity(nc, ident)
        iota_i = const.tile([P, KC], i32, tag="iota_i")
        nc.gpsimd.iota(iota_i[:], pattern=[[P, KC]], base=0, channel_multiplier=1)
        iota_f = const.tile([P, KC], f32, tag="iota_f")
        nc.vector.tensor_copy(iota_f[:], iota_i[:])

        cb_sb = const.tile([P, TC], f32, tag="cb_sb")
        nc.sync.dma_start(cb_sb[:], cb_d.rearrange("(jc p) -> p jc", p=P))
        near_row = const.tile([1, S], f32, tag="near_row")
        nc.sync.dma_start(near_row[:], nearf_d[None, :])
        near_bc = const.tile([P, S], f32, tag="near_bc")
        nc.gpsimd.partition_broadcast(near_bc[:], near_row[:])

        bq_sb = const.tile([P, FC], f32, tag="bq_sb")
        nc.sync.dma_start(bq_sb[:], bqs_d.rearrange("(fc p) -> p fc", p=P))
        bk_sb = const.tile([P, FC], f32, tag="bk_sb")
        nc.sync.dma_start(bk_sb[:], bk_d.rearrange("(fc p) -> p fc", p=P))
        bv_row = const.tile([1, F], f32, tag="bv_row")
        nc.sync.dma_start(bv_row[:], bv_d[None, :])
        bv_bc = const.tile([P, F], f32, tag="bv_bc")
        nc.gpsimd.partition_broadcast(bv_bc[:], bv_row[:])

        qT = big.tile([P, FC, S], f32r, tag="qT")
        kT = big.tile([P, FC, S], f32r, tag="kT")
        v_sb = big.tile([P, TC, HPC, 65], f32r, tag="v_sb")
        ones64 = const.tile([P, TC * HPC], f32, tag="ones64")
        nc.vector.memset(ones64[:], 1.0)
        nc.vector.tensor_copy(
            v_sb[:, :, :, 64:65],
            ones64.rearrange("p (a b one) -> p a b one", a=TC, b=HPC, one=1),
        )

        projpool = ctx.enter_context(tc.tile_pool(name="projpool", bufs=1))
        wq_sb = projpool.tile([P, KC, F], f32r, tag="wq_sb")
        xTh = []
        for i in range(IC):
            xthalf = projpool.tile([P, KC, 512], f32r, tag=f"xT{i}", name=f"xT{i}")
            xTh.append(xthalf)

        # ---- transposes + v/k projections (wk/wv freed after) ----
        with tc.tile_pool(name="wkvpool", bufs=1) as wkvpool, \
             tc.tile_pool(name="xstage", bufs=4) as xstage:
            wk_sb = wkvpool.tile([P, KC, F], f32r, tag="wk_sb")
            wv_sb = wkvpool.tile([P, KC, F], f32r, tag="wv_sb")
            # x DMAs + transposes first; weights behind them on the DMA engine
            def emit_vproj(tci_range):
                for tci in tci_range:
                    ps = ps_q.tile([P, 512], f32, tag="ps_proj")
                    for kc in range(KC):
                        nc.tensor.matmul(
                            ps[:],
                            xTh[tci // 4][:, kc, (tci % 4) * P : (tci % 4 + 1) * P],
                            wv_sb[:, kc, :],
                            start=(kc == 0), stop=(kc == KC - 1),
                        )
                    nc.vector.tensor_tensor(
                        v_sb[:, tci, :, 0:64],
                        ps.rearrange("p (h d) -> p h d", d=64),
                        bv_bc.rearrange("p (h d) -> p h d", d=64),
                        mybir.AluOpType.add,
                    )

            for half in range(IC):
                xts = []
                for k in range(4):
                    xt = xstage.tile([P, H], f32, tag="xt", name=f"xt_{half}_{k}")
                    nc.sync.dma_start(
                        xt[:], x_d[(half * 4 + k) * P : (half * 4 + k + 1) * P, :]
                    )
                    xts.append(xt)
                if half == 0:
                    nc.sync.dma_start(wv_sb[:], wv_d.bitcast(f32r).rearrange("(kc p) f -> p kc f", p=P))
                xdst = xTh[half]
                for hc in range(KC):
                    pst = ps_q.tile([P, 512], f32, tag="ps_proj")
                    for k in range(4):
                        nc.tensor.transpose(
                            pst[:, k * P : (k + 1) * P],
                            xts[k][:, hc * P : (hc + 1) * P], ident[:],
                        )
                    if hc % 2 == 0:
                        nc.vector.tensor_copy(xdst[:, hc, :], pst[:])
                    else:
                        nc.scalar.copy(xdst[:, hc, :], pst[:])
                if half == 0:
                    # v-proj of the first token half fills PE while the second
                    # x half is still in flight on the DMA engine
                    emit_vproj(range(0, 4))
            nc.sync.dma_start(wk_sb[:], wk_d.bitcast(f32r).rearrange("(kc p) f -> p kc f", p=P))
            nc.sync.dma_start(wq_sb[:], wq_d.bitcast(f32r).rearrange("(kc p) f -> p kc f", p=P))
            emit_vproj(range(4, 8))
            # k projection
            for fc in range(FC):
                for ic in range(IC):
                    ps = ps_q.tile([P, 512], f32, tag="ps_proj")
                    for kc in range(KC):
                        nc.tensor.matmul(
                            ps[:],
                            wk_sb[:, kc, fc * P : (fc + 1) * P],
                            xTh[ic][:, kc, :],
                            start=(kc == 0), stop=(kc == KC - 1),
                        )
                    nc.scalar.activation(
                        kT[:, fc, ic * 512 : (ic + 1) * 512], ps[:],
                        mybir.ActivationFunctionType.Identity,
                        bias=bk_sb[:, fc : fc + 1], scale=1.0,
                    )

        # ---- attention interleaved with q projection, per head pair ----
        attn2 = ctx.enter_context(tc.tile_pool(name="attn2", bufs=1))
        ohstage = ctx.enter_context(tc.tile_pool(name="ohstage", bufs=2))
        # verb factor (transposed): ebT[p, jc, i] = exp(2 * (jc*128+p == nearest[i]))
        ebT = attn2.tile([P, TC, S], bf16, tag="ebT")
        for jc in range(TC):
            ohst = ohstage.tile([P, S], f32, tag="ohst")
            nc.vector.tensor_scalar(
                ohst[:], near_bc[:], iota_f[:, jc : jc + 1], 2.0,
                mybir.AluOpType.is_equal, mybir.AluOpType.mult,
            )
            nc.scalar.activation(
                ebT[:, jc, :], ohst[:], mybir.ActivationFunctionType.Exp
            )

        # head h -> partitions (h%2)*64.., feature chunk h//2 (pairs stacked on
        # the partition axis so o_proj contracts K=128)
        oT = attn2.tile([P, FC, S], bf16, tag="oT")
        wo_sb = attn2.tile([P, FC, H], bf16, tag="wo_sb")
        nc.gpsimd.dma_start(wo_sb[:], wo_d.rearrange("(fc p) o -> p fc o", p=P))

        def emit_oproj(tci_range):
            for tci in tci_range:
                for oc in range(IC):
                    psz = ps_q.tile([P, 512], f32, tag="ps_proj")
                    for fc in range(FC):
                        nc.tensor.matmul(
                            psz[:],
                            oT[:, fc, tci * P : (tci + 1) * P],
                            wo_sb[:, fc, oc * 512 : (oc + 1) * 512],
                            start=(fc == 0), stop=(fc == FC - 1),
                        )
                    zt = zpool.tile([P, 512], f32, tag="zt")
                    if (tci + oc) % 2 == 0:
                        nc.scalar.copy(zt[:], psz[:])
                    else:
                        nc.vector.tensor_copy(zt[:], psz[:])
                    nc.sync.dma_start(z_d[tci * P : (tci + 1) * P, oc * 512 : (oc + 1) * 512], zt[:])

        def flush_divisions(pending):
            for (ic_, fc4_, side_, pso_, rlb_) in pending:
                hb = side_ * 64
                nc.vector.tensor_tensor(
                    oT[hb : hb + 64, fc4_, ic_ * 512 : (ic_ + 1) * 512],
                    pso_[0:64, :], rlb_[:],
                    mybir.AluOpType.mult,
                )
            pending.clear()

        pending = []
        ic0_done = False
        for fc4 in range(FC):
            # q projection for this pair's feature chunk
            for ic in range(IC):
                ps = ps_q.tile([P, 512], f32, tag="ps_proj")
                for kc in range(KC):
                    nc.tensor.matmul(
                        ps[:],
                        wq_sb[:, kc, fc4 * P : (fc4 + 1) * P],
                        xTh[ic][:, kc, :],
                        start=(kc == 0), stop=(kc == KC - 1),
                    )
                nc.scalar.activation(
                    qT[:, fc4, ic * 512 : (ic + 1) * 512], ps[:],
                    mybir.ActivationFunctionType.Identity,
                    bias=bq_sb[:, fc4 : fc4 + 1], scale=SCALE,
                )
            for ic in range(IC):
                # heads of the pair interleaved: PE alternates A/B matmuls
                # while ACT/DVE process the other head's exp / verb multiply
                psos = []
                for side in range(2):
                    pso = ps_o.tile([65, 512], f32, tag="pso", name=f"pso_{side}")
                    psos.append(pso)
                for jc in range(TC):
                    pTs = []
                    for side in range(2):
                        hb = side * 64
                        pss = ps_s.tile([P, 512], f32, tag="pss", name=f"pss_{side}")
                        nc.tensor.matmul(
                            pss[:],
                            kT[hb : hb + 64, fc4, jc * P : (jc + 1) * P],
                            qT[hb : hb + 64, fc4, ic * 512 : (ic + 1) * 512],
                            start=True, stop=True,
                        )
                        pT = ppool.tile([P, 512], f32r, tag="pT", name=f"pT_{side}")
                        nc.scalar.activation(
                            pT[:], pss[:], mybir.ActivationFunctionType.Exp,
                            bias=cb_sb[:, jc : jc + 1], scale=1.0,
                        )
                        mul_eng = nc.gpsimd if (jc % 2 == 1 and side == 1) else nc.vector
                        mul_eng.tensor_tensor(
                            pT[:], pT[:], ebT[:, jc, ic * 512 : (ic + 1) * 512],
                            mybir.AluOpType.mult,
                        )
                        pTs.append(pT)
                    for side in range(2):
                        h = 2 * fc4 + side
                        nc.tensor.matmul(
                            psos[side][:],
                            v_sb[:, jc, h, 0:65],
                            pTs[side][:],
                            start=(jc == 0), stop=(jc == TC - 1),
                        )
                # previous group's divisions (their broadcasts completed while
                # this group was streaming) — keeps the DVE stream stall-free
                flush_divisions(pending)
                if fc4 == FC - 1 and ic == 1 and not ic0_done:
                    # all ic=0 oT divided once (fc3, ic0)'s flush ran above
                    emit_oproj(range(0, 4))
                    ic0_done = True
                for side in range(2):
                    pso = psos[side]
                    rl = rlpool.tile([65, 512], f32, tag="rl")
                    nc.vector.reciprocal(rl[64:65, :], pso[64:65, :])
                    rlrow = rlpool.tile([1, 512], f32, tag="rlrow")
                    nc.sync.dma_start(rlrow[:], rl[64:65, :])
                    rlb = rlpool.tile([64, 512], f32, tag="rlb")
                    nc.gpsimd.partition_broadcast(rlb[:], rlrow[:])
                    pending.append((ic, fc4, side, pso, rlb))
        flush_divisions(pending)
        emit_oproj(range(4, 8))

    nc.compile()
    return nc


def _get_compiled():
    global _COMPILED
    if _COMPILED is None:
        _COMPILED = _build()
    return _COMPILED


def _host_morpho(morpho_types):
    """nearest-verb index per (b, i) (-1 if batch has no verb) and col bias."""
    mt = np.asarray(morpho_types)
    pos = np.arange(S)
    dist = np.abs(pos[:, None] - pos[None, :]).astype(np.float32)
    nearest = np.empty((B, S), np.float32)
    for b in range(B):
        is_verb = mt[b] == 2
        if not is_verb.any():
            nearest[b] = -1.0
            continue
        dm = np.where(is_verb[None, :], dist, BIG)
        nearest[b] = np.argmin(dm, axis=-1).astype(np.float32)
    cb = (
        np.float32(ROOT_BIAS * 0.5) * (mt == 0)
        + np.float32(SUFFIX_BIAS * 0.3) * (mt == 1)
    ).astype(np.float32)
    return nearest, cb


def kernel(hidden_states, morpho_types, Wq, bq, Wk, bk, Wv, bv, Wo, bo):
    hidden_states = np.ascontiguousarray(np.asarray(hidden_states, np.float32))
    Wq = np.asarray(Wq, np.float32)
    Wk = np.asarray(Wk, np.float32)
    Wv = np.asarray(Wv, np.float32)
    Wo = np.asarray(Wo, np.float32)
    bq = np.asarray(bq, np.float32)
    bk = np.asarray(bk, np.float32)
    bv = np.asarray(bv, np.float32)
    bo = np.asarray(bo, np.float32)

    nearest, cb = _host_morpho(morpho_types)

    nc = _get_compiled()
    in_maps = []
    for c in range(8):
        b, g = c // G, c % G
        fs = slice(g * F, (g + 1) * F)
        in_maps.append({
            "x": hidden_states[b],
            "wq": np.ascontiguousarray(Wq[:, fs]),
            "wk": np.ascontiguousarray(Wk[:, fs]),
            "wv": np.ascontiguousarray(Wv[:, fs]),
            "wo": np.ascontiguousarray(Wo[fs, :]),
            "bqs": np.ascontiguousarray(bq[fs]) * np.float32(SCALE),
            "bk": np.ascontiguousarray(bk[fs]),
            "bv": np.ascontiguousarray(bv[fs]),
            "nearf": nearest[b],
            "cb": cb[b],
        })

    res = run_bass_kernel_spmd(nc, in_maps, core_ids=list(range(8)))
    out = np.empty((B, S, H), np.float32)
    for b in range(B):
        out[b] = res.results[2 * b]["z"] + res.results[2 * b + 1]["z"] + bo
    return out


# revision 22
# speedup vs baseline: 86.8900x; 1.0493x over previous
"""AgglutinativeAttention Trainium2 kernel.

Full inputs in, full output out. Sharding: 8 cores = (batch b in 0..3) x
(head-group g in 0..1). Each core computes, for its batch b and its 8 heads:
  qT/kT = (x @ W{q,k}[:, gF:(g+1)F])^T   [512 feat, 1024 tok]  (q pre-scaled)
  v     =  x @ Wv[:, gF:(g+1)F]          [1024 tok, 512 feat] (+ones col/head)
  per head: sT = kT_h^T-style scores transposed [j, i] via PE,
  morpho bias injected via identity-matmul (verb one-hot) + per-partition
  activation bias (col bias), pT = exp(sT + bias), oT = v_aug^T @ pT with a
  ones row giving the softmax denominator, divide, then partial
  z = o @ Wo[gF:(g+1)F, :].  Host sums the two per-batch partials + bo.
"""

import numpy as np
from contextlib import ExitStack

import concourse.bass as bass
import concourse.mybir as mybir
import concourse.tile as tile
from concourse import bacc
from concourse.bass_utils import run_bass_kernel_spmd
from concourse.masks import make_identity

B, S, H = 4, 1024, 1024
NH, HD = 16, 64
G = 2                 # head groups (tensor-parallel factor per batch)
F = H // G            # 512 features per core
HPC = NH // G         # 8 heads per core
SCALE = 1.0 / np.sqrt(HD)
VERB_BIAS, ROOT_BIAS, SUFFIX_BIAS = 2.0, 1.5, 1.2
BIG = np.float32(1e9)

f32 = mybir.dt.float32
f32r = mybir.dt.float32r
i32 = mybir.dt.int32

P = 128
KC = H // P           # 8 contraction chunks for projections
TC = S // P           # 8 token chunks of 128
IC = S // 512         # 2 chunks of 512 (matmul free dim)
FC = F // P           # 4 feature chunks per core

_COMPILED = None


def _build():
    nc = bacc.Bacc("TRN2", target_bir_lowering=False, debug=False, num_devices=8)

    x_d = nc.dram_tensor("x", [S, H], f32, kind="ExternalInput").ap()
    wq_d = nc.dram_tensor("wq", [H, F], f32, kind="ExternalInput").ap()
    wk_d = nc.dram_tensor("wk", [H, F], f32, kind="ExternalInput").ap()
    wv_d = nc.dram_tensor("wv", [H, F], f32, kind="ExternalInput").ap()
    wo_d = nc.dram_tensor("wo", [F, H], f32, kind="ExternalInput").ap()
    bqs_d = nc.dram_tensor("bqs", [F], f32, kind="ExternalInput").ap()
    bk_d = nc.dram_tensor("bk", [F], f32, kind="ExternalInput").ap()
    bv_d = nc.dram_tensor("bv", [F], f32, kind="ExternalInput").ap()
    nearf_d = nc.dram_tensor("nearf", [S], f32, kind="ExternalInput").ap()
    cb_d = nc.dram_tensor("cb", [S], f32, kind="ExternalInput").ap()
    z_d = nc.dram_tensor("z", [S, H], f32, kind="ExternalOutput").ap()

    with tile.TileContext(nc) as tc, ExitStack() as ctx:
        const = ctx.enter_context(tc.tile_pool(name="const", bufs=1))
        big = ctx.enter_context(tc.tile_pool(name="big", bufs=1))
        ppool = ctx.enter_context(tc.tile_pool(name="ppool", bufs=6))
        rlpool = ctx.enter_context(tc.tile_pool(name="rlpool", bufs=4))
        zpool = ctx.enter_context(tc.tile_pool(name="zpool", bufs=3))
        ps_q = ctx.enter_context(tc.tile_pool(name="ps_q", bufs=2, space="PSUM"))
        ps_s = ctx.enter_context(tc.tile_pool(name="ps_s", bufs=2, space="PSUM"))
        ps_o = ctx.enter_context(tc.tile_pool(name="ps_o", bufs=4, space="PSUM"))

        # ---- constants ----
        ident = const.tile([P, P], f32, tag="ident")
        make_identity(nc, ident)
        iota_i = const.tile([P, KC], i32, tag="iota_i")
        nc.gpsimd.iota(iota_i[:], pattern=[[P, KC]], base=0, channel_multiplier=1)
        iota_f = const.tile([P, KC], f32, tag="iota_f")
        nc.vector.tensor_copy(iota_f[:], iota_i[:])

        cb_sb = const.tile([P, TC], f32, tag="cb_sb")
        nc.sync.dma_start(cb_sb[:], cb_d.rearrange("(jc p) -> p jc", p=P))
        near_row = const.tile([1, S], f32, tag="near_row")
        nc.sync.dma_start(near_row[:], nearf_d[None, :])
        near_bc = const.tile([P, S], f32, tag="near_bc")
        nc.gpsimd.partition_broadcast(near_bc[:], near_row[:])

        bq_sb = const.tile([P, FC], f32, tag="bq_sb")
        nc.sync.dma_start(bq_sb[:], bqs_d.rearrange("(fc p) -> p fc", p=P))
        bk_sb = const.tile([P, FC], f32, tag="bk_sb")
        nc.sync.dma_start(bk_sb[:], bk_d.rearrange("(fc p) -> p fc", p=P))
        bv_row = const.tile([1, F], f32, tag="bv_row")
        nc.sync.dma_start(bv_row[:], bv_d[None, :])
        bv_bc = const.tile([P, F], f32, tag="bv_bc")
        nc.gpsimd.partition_broadcast(bv_bc[:], bv_row[:])

        bf16 = mybir.dt.bfloat16
        qT = big.tile([P, FC, S], f32r, tag="qT")
        kT = big.tile([P, FC, S], f32r, tag="kT")
        v_sb = big.tile([P, TC, HPC, 65], bf16, tag="v_sb")
        ones64 = const.tile([P, TC * HPC], f32, tag="ones64")
        nc.vector.memset(ones64[:], 1.0)
        nc.vector.tensor_copy(
            v_sb[:, :, :, 64:65],
            ones64.rearrange("p (a b one) -> p a b one", a=TC, b=HPC, one=1),
        )

        # verb factor (transposed): ebT[p, jc, i] = exp(2 * (jc*128+p == nearest[i]))
        # computed up front so its ACT work doesn't sit between the k-proj and
        # q-proj copies in the (in-order) ACT stream
        ebT = big.tile([P, TC, S], bf16, tag="ebT")
        ohstage = ctx.enter_context(tc.tile_pool(name="ohstage", bufs=1))
        for jc in range(TC):
            ohst = ohstage.tile([P, S], f32, tag="ohst")
            nc.vector.tensor_scalar(
                ohst[:], near_bc[:], iota_f[:, jc : jc + 1], 2.0,
                mybir.AluOpType.is_equal, mybir.AluOpType.mult,
            )
            nc.scalar.activation(
                ebT[:, jc, :], ohst[:], mybir.ActivationFunctionType.Exp
            )

        projpool = ctx.enter_context(tc.tile_pool(name="projpool", bufs=1))
        wq_sb = projpool.tile([P, KC, F], f32r, tag="wq_sb")
        xTh = []
        for i in range(IC):
            xthalf = projpool.tile([P, KC, 512], f32r, tag=f"xT{i}", name=f"xT{i}")
            xTh.append(xthalf)

        # ---- transposes + v/k projections (wk/wv freed after) ----
        with tc.tile_pool(name="wkvpool", bufs=1) as wkvpool, \
             tc.tile_pool(name="xstage", bufs=4) as xstage:
            wk_sb = wkvpool.tile([P, KC, F], f32r, tag="wk_sb")
            wv_sb = wkvpool.tile([P, KC, F], f32r, tag="wv_sb")
            # x DMAs + transposes first; weights behind them on the DMA engine
            def emit_vproj(tci_range):
                for tci in tci_range:
                    ps = ps_q.tile([P, 512], f32, tag="ps_proj")
                    for kc in range(KC):
                        nc.tensor.matmul(
                            ps[:],
                            xTh[tci // 4][:, kc, (tci % 4) * P : (tci % 4 + 1) * P],
                            wv_sb[:, kc, :],
                            start=(kc == 0), stop=(kc == KC - 1),
                        )
                    nc.vector.tensor_tensor(
                        v_sb[:, tci, :, 0:64],
                        ps.rearrange("p (h d) -> p h d", d=64),
                        bv_bc.rearrange("p (h d) -> p h d", d=64),
                        mybir.AluOpType.add,
                    )

            for half in range(IC):
                xts = []
                for k in range(4):
                    xt = xstage.tile([P, H], f32, tag="xt", name=f"xt_{half}_{k}")
                    nc.sync.dma_start(
                        xt[:], x_d[(half * 4 + k) * P : (half * 4 + k + 1) * P, :]
                    )
                    xts.append(xt)
                if half == 0:
                    nc.sync.dma_start(wv_sb[:], wv_d.bitcast(f32r).rearrange("(kc p) f -> p kc f", p=P))
                xdst = xTh[half]
                for hc in range(KC):
                    pst = ps_q.tile([P, 512], f32, tag="ps_proj")
                    for k in range(4):
                        nc.tensor.transpose(
                            pst[:, k * P : (k + 1) * P],
                            xts[k][:, hc * P : (hc + 1) * P], ident[:],
                        )
                    if hc % 2 == 0:
                        nc.vector.tensor_copy(xdst[:, hc, :], pst[:])
                    else:
                        nc.scalar.copy(xdst[:, hc, :], pst[:])
                if half == 0:
                    # v-proj of the first token half fills PE while the second
                    # x half is still in flight on the DMA engine
                    emit_vproj(range(0, 4))
            nc.sync.dma_start(wk_sb[:], wk_d.bitcast(f32r).rearrange("(kc p) f -> p kc f", p=P))
            nc.sync.dma_start(wq_sb[:], wq_d.bitcast(f32r).rearrange("(kc p) f -> p kc f", p=P))
            emit_vproj(range(4, 8))
            # k projection
            for fc in range(FC):
                for ic in range(IC):
                    ps = ps_q.tile([P, 512], f32, tag="ps_proj")
                    for kc in range(KC):
                        nc.tensor.matmul(
                            ps[:],
                            wk_sb[:, kc, fc * P : (fc + 1) * P],
                            xTh[ic][:, kc, :],
                            start=(kc == 0), stop=(kc == KC - 1),
                        )
                    nc.scalar.activation(
                        kT[:, fc, ic * 512 : (ic + 1) * 512], ps[:],
                        mybir.ActivationFunctionType.Identity,
                        bias=bk_sb[:, fc : fc + 1], scale=1.0,
                    )

        # ---- attention interleaved with q projection, per head pair ----
        attn2 = ctx.enter_context(tc.tile_pool(name="attn2", bufs=1))

        # head h -> partitions (h%2)*64.., feature chunk h//2 (pairs stacked on
        # the partition axis so o_proj contracts K=128)
        oT = attn2.tile([P, FC, S], bf16, tag="oT")
        wo_sb = attn2.tile([P, FC, H], bf16, tag="wo_sb")
        nc.gpsimd.dma_start(wo_sb[:], wo_d.rearrange("(fc p) o -> p fc o", p=P))

        def emit_oproj(tci_range):
            for tci in tci_range:
                for oc in range(IC):
                    psz = ps_q.tile([P, 512], f32, tag="ps_proj")
                    for fc in range(FC):
                        nc.tensor.matmul(
                            psz[:],
                            oT[:, fc, tci * P : (tci + 1) * P],
                            wo_sb[:, fc, oc * 512 : (oc + 1) * 512],
                            start=(fc == 0), stop=(fc == FC - 1),
                        )
                    zt = zpool.tile([P, 512], f32, tag="zt")
                    if (tci + oc) % 2 == 0:
                        nc.scalar.copy(zt[:], psz[:])
                    else:
                        nc.vector.tensor_copy(zt[:], psz[:])
                    nc.sync.dma_start(z_d[tci * P : (tci + 1) * P, oc * 512 : (oc + 1) * 512], zt[:])

        def flush_divisions(pending):
            for (ic_, fc4_, side_, pso_, rlb_) in pending:
                hb = side_ * 64
                nc.vector.tensor_tensor(
                    oT[hb : hb + 64, fc4_, ic_ * 512 : (ic_ + 1) * 512],
                    pso_[0:64, :], rlb_[:],
                    mybir.AluOpType.mult,
                )
            pending.clear()

        pending = []
        ic0_done = False
        for fc4 in range(FC):
            # q projection for this pair's feature chunk
            for ic in range(IC):
                ps = ps_q.tile([P, 512], f32, tag="ps_proj")
                for kc in range(KC):
                    nc.tensor.matmul(
                        ps[:],
                        wq_sb[:, kc, fc4 * P : (fc4 + 1) * P],
                        xTh[ic][:, kc, :],
                        start=(kc == 0), stop=(kc == KC - 1),
                    )
                nc.scalar.activation(
                    qT[:, fc4, ic * 512 : (ic + 1) * 512], ps[:],
                    mybir.ActivationFunctionType.Identity,
                    bias=bq_sb[:, fc4 : fc4 + 1], scale=SCALE,
                )
            for ic in range(IC):
                # heads of the pair interleaved: PE alternates A/B matmuls
                # while ACT/DVE process the other head's exp / verb multiply
                psos = []
                for side in range(2):
                    pso = ps_o.tile([65, 512], f32, tag="pso", name=f"pso_{side}")
                    psos.append(pso)
                for jc in range(TC):
                    pTs = []
                    for side in range(2):
                        hb = side * 64
                        pss = ps_s.tile([P, 512], f32, tag="pss", name=f"pss_{side}")
                        nc.tensor.matmul(
                            pss[:],
                            kT[hb : hb + 64, fc4, jc * P : (jc + 1) * P],
                            qT[hb : hb + 64, fc4, ic * 512 : (ic + 1) * 512],
                            start=True, stop=True,
                        )
                        pT = ppool.tile([P, 512], bf16, tag="pT", name=f"pT_{side}")
                        nc.scalar.activation(
                            pT[:], pss[:], mybir.ActivationFunctionType.Exp,
                            bias=cb_sb[:, jc : jc + 1], scale=1.0,
                        )
                        mul_eng = nc.gpsimd if (jc % 2 == 1 and side == 1) else nc.vector
                        mul_eng.tensor_tensor(
                            pT[:], pT[:], ebT[:, jc, ic * 512 : (ic + 1) * 512],
                            mybir.AluOpType.mult,
                        )
                        pTs.append(pT)
                    for side in range(2):
                        h = 2 * fc4 + side
                        nc.tensor.matmul(
                            psos[side][:],
                            v_sb[:, jc, h, 0:65],
                            pTs[side][:],
                            start=(jc == 0), stop=(jc == TC - 1),
                        )
                # previous group's divisions (their broadcasts completed while
                # this group was streaming) — keeps the DVE stream stall-free
                flush_divisions(pending)
                if fc4 == FC - 1 and ic == 1 and not ic0_done:
                    # all ic=0 oT divided once (fc3, ic0)'s flush ran above
                    emit_oproj(range(0, 4))
                    ic0_done = True
                for side in range(2):
                    pso = psos[side]
                    rl = rlpool.tile([65, 512], f32, tag="rl")
                    nc.vector.reciprocal(rl[64:65, :], pso[64:65, :])
                    rlrow = rlpool.tile([1, 512], f32, tag="rlrow")
                    nc.sync.dma_start(rlrow[:], rl[64:65, :])
                    rlb = rlpool.tile([64, 512], f32, tag="rlb")
                    nc.gpsimd.partition_broadcast(rlb[:], rlrow[:])
                    pending.append((ic, fc4, side, pso, rlb))
        flush_divisions(pending)
        emit_oproj(range(4, 8))

    nc.compile()
    return nc


def _get_compiled():
    global _COMPILED
    if _COMPILED is None:
        _COMPILED = _build()
    return _COMPILED


def _host_morpho(morpho_types):
    """nearest-verb index per (b, i) (-1 if batch has no verb) and col bias."""
    mt = np.asarray(morpho_types)
    pos = np.arange(S)
    dist = np.abs(pos[:, None] - pos[None, :]).astype(np.float32)
    nearest = np.empty((B, S), np.float32)
    for b in range(B):
        is_verb = mt[b] == 2
        if not is_verb.any():
            nearest[b] = -1.0
            continue
        dm = np.where(is_verb[None, :], dist, BIG)
        nearest[b] = np.argmin(dm, axis=-1).astype(np.float32)
    cb = (
        np.float32(ROOT_BIAS * 0.5) * (mt == 0)
        + np.float32(SUFFIX_BIAS * 0.3) * (mt == 1)
    ).astype(np.float32)
    return nearest, cb


def kernel(hidden_states, morpho_types, Wq, bq, Wk, bk, Wv, bv, Wo, bo):
    hidden_states = np.ascontiguousarray(np.asarray(hidden_states, np.float32))
    Wq = np.asarray(Wq, np.float32)
    Wk = np.asarray(Wk, np.float32)
    Wv = np.asarray(Wv, np.float32)
    Wo = np.asarray(Wo, np.float32)
    bq = np.asarray(bq, np.float32)
    bk = np.asarray(bk, np.float32)
    bv = np.asarray(bv, np.float32)
    bo = np.asarray(bo, np.float32)

    nearest, cb = _host_morpho(morpho_types)

    nc = _get_compiled()
    in_maps = []
    for c in range(8):
        b, g = c // G, c % G
        fs = slice(g * F, (g + 1) * F)
        in_maps.append({
            "x": hidden_states[b],
            "wq": np.ascontiguousarray(Wq[:, fs]),
            "wk": np.ascontiguousarray(Wk[:, fs]),
            "wv": np.ascontiguousarray(Wv[:, fs]),
            "wo": np.ascontiguousarray(Wo[fs, :]),
            "bqs": np.ascontiguousarray(bq[fs]) * np.float32(SCALE),
            "bk": np.ascontiguousarray(bk[fs]),
            "bv": np.ascontiguousarray(bv[fs]),
            "nearf": nearest[b],
            "cb": cb[b],
        })

    res = run_bass_kernel_spmd(nc, in_maps, core_ids=list(range(8)))
    out = np.empty((B, S, H), np.float32)
    for b in range(B):
        out[b] = res.results[2 * b]["z"] + res.results[2 * b + 1]["z"] + bo
    return out


# revision 38
# speedup vs baseline: 98.0393x; 1.1283x over previous
"""AgglutinativeAttention Trainium2 kernel.

Full inputs in, full output out. Sharding: 8 cores = (batch b in 0..3) x
(head-group g in 0..1). Each core computes, for its batch b and its 8 heads:
  qT/kT = (x @ W{q,k}[:, gF:(g+1)F])^T   [512 feat, 1024 tok]  (q pre-scaled)
  v     =  x @ Wv[:, gF:(g+1)F]          [1024 tok, 512 feat] (+ones col/head)
  per head: sT = kT_h^T-style scores transposed [j, i] via PE,
  morpho bias injected via identity-matmul (verb one-hot) + per-partition
  activation bias (col bias), pT = exp(sT + bias), oT = v_aug^T @ pT with a
  ones row giving the softmax denominator, divide, then partial
  z = o @ Wo[gF:(g+1)F, :].  Host sums the two per-batch partials + bo.
"""

import numpy as np
from contextlib import ExitStack

import concourse.bass as bass
import concourse.mybir as mybir
import concourse.tile as tile
from concourse import bacc
from concourse.bass_utils import run_bass_kernel_spmd
from concourse.masks import make_identity

B, S, H = 4, 1024, 1024
NH, HD = 16, 64
G = 2                 # head groups (tensor-parallel factor per batch)
F = H // G            # 512 features per core
HPC = NH // G         # 8 heads per core
SCALE = 1.0 / np.sqrt(HD)
VERB_BIAS, ROOT_BIAS, SUFFIX_BIAS = 2.0, 1.5, 1.2
BIG = np.float32(1e9)

f32 = mybir.dt.float32
f32r = mybir.dt.float32r
i32 = mybir.dt.int32

P = 128
KC = H // P           # 8 contraction chunks for projections
TC = S // P           # 8 token chunks of 128
IC = S // 512         # 2 chunks of 512 (matmul free dim)
FC = F // P           # 4 feature chunks per core

_COMPILED = None


def _build():
    nc = bacc.Bacc("TRN2", target_bir_lowering=False, debug=False, num_devices=8)

    x_d = nc.dram_tensor("x", [S, H], f32, kind="ExternalInput").ap()
    wq_d = nc.dram_tensor("wq", [H, F], f32, kind="ExternalInput").ap()
    wk_d = nc.dram_tensor("wk", [H, F], f32, kind="ExternalInput").ap()
    wv_d = nc.dram_tensor("wv", [H, F], f32, kind="ExternalInput").ap()
    wo_d = nc.dram_tensor("wo", [F, H], f32, kind="ExternalInput").ap()
    bqs_d = nc.dram_tensor("bqs", [F], f32, kind="ExternalInput").ap()
    bk_d = nc.dram_tensor("bk", [F], f32, kind="ExternalInput").ap()
    bv_d = nc.dram_tensor("bv", [F], f32, kind="ExternalInput").ap()
    nearf_d = nc.dram_tensor("nearf", [S], f32, kind="ExternalInput").ap()
    cb_d = nc.dram_tensor("cb", [S], f32, kind="ExternalInput").ap()
    z_d = nc.dram_tensor("z", [S, H], f32, kind="ExternalOutput").ap()

    with tile.TileContext(nc) as tc, ExitStack() as ctx:
        const = ctx.enter_context(tc.tile_pool(name="const", bufs=1))
        big = ctx.enter_context(tc.tile_pool(name="big", bufs=1))
        ppool = ctx.enter_context(tc.tile_pool(name="ppool", bufs=6))
        rlpool = ctx.enter_context(tc.tile_pool(name="rlpool", bufs=4))
        zpool = ctx.enter_context(tc.tile_pool(name="zpool", bufs=3))
        ps_q = ctx.enter_context(tc.tile_pool(name="ps_q", bufs=2, space="PSUM"))
        ps_s = ctx.enter_context(tc.tile_pool(name="ps_s", bufs=2, space="PSUM"))
        ps_o = ctx.enter_context(tc.tile_pool(name="ps_o", bufs=4, space="PSUM"))

        # ---- constants ----
        ident = const.tile([P, P], f32, tag="ident")
        make_identity(nc, ident)
        iota_i = const.tile([P, KC], i32, tag="iota_i")
        nc.gpsimd.iota(iota_i[:], pattern=[[P, KC]], base=0, channel_multiplier=1)
        iota_f = const.tile([P, KC], f32, tag="iota_f")
        nc.vector.tensor_copy(iota_f[:], iota_i[:])

        near_row = const.tile([1, S], f32, tag="near_row")
        nc.sync.dma_start(near_row[:], nearf_d[None, :])
        near_bc = const.tile([P, S], f32, tag="near_bc")
        nc.gpsimd.partition_broadcast(near_bc[:], near_row[:])
        bv_row = const.tile([1, F], f32, tag="bv_row")
        nc.sync.dma_start(bv_row[:], bv_d[None, :])
        bv_bc = const.tile([P, F], f32, tag="bv_bc")
        nc.gpsimd.partition_broadcast(bv_bc[:], bv_row[:])
        cb_sb = const.tile([P, TC], f32, tag="cb_sb")
        bq_sb = const.tile([P, FC], f32, tag="bq_sb")
        bk_sb = const.tile([P, FC], f32, tag="bk_sb")

        bf16 = mybir.dt.bfloat16
        qT = big.tile([P, FC, S], f32r, tag="qT")
        kT = big.tile([P, FC, S], f32r, tag="kT")
        v_sb = big.tile([P, TC, HPC, 65], bf16, tag="v_sb")
        ones64 = const.tile([P, TC * HPC], f32, tag="ones64")
        nc.vector.memset(ones64[:], 1.0)
        nc.vector.tensor_copy(
            v_sb[:, :, :, 64:65],
            ones64.rearrange("p (a b one) -> p a b one", a=TC, b=HPC, one=1),
        )

        # verb factor (transposed): ebT[p, jc, i] = exp(2 * (jc*128+p == nearest[i]))
        # computed up front so its ACT work doesn't sit between the k-proj and
        # q-proj copies in the (in-order) ACT stream
        ebT = big.tile([P, TC, S], bf16, tag="ebT")
        ohstage = ctx.enter_context(tc.tile_pool(name="ohstage", bufs=1))
        for jc in range(TC):
            ohst = ohstage.tile([P, S], f32, tag="ohst")
            nc.vector.tensor_scalar(
                ohst[:], near_bc[:], iota_f[:, jc : jc + 1], 2.0,
                mybir.AluOpType.is_equal, mybir.AluOpType.mult,
            )
            nc.scalar.activation(
                ebT[:, jc, :], ohst[:], mybir.ActivationFunctionType.Exp
            )

        projpool = ctx.enter_context(tc.tile_pool(name="projpool", bufs=1))
        wq_sb = projpool.tile([P, KC, F], f32r, tag="wq_sb")
        wk_sb = projpool.tile([P, KC, F], f32r, tag="wk_sb")
        xTh = []
        for i in range(IC):
            xthalf = projpool.tile([P, KC, 512], f32r, tag=f"xT{i}", name=f"xT{i}")
            xTh.append(xthalf)

        # ---- transposes + v/k projections (wk/wv freed after) ----
        with tc.tile_pool(name="wkvpool", bufs=1) as wkvpool, \
             tc.tile_pool(name="xstage", bufs=8) as xstage:
            wv_sb = wkvpool.tile([P, KC, F], f32r, tag="wv_sb")
            # x DMAs + transposes first; weights behind them on the DMA engine
            def emit_vproj(tci_range):
                for tci in tci_range:
                    ps = ps_q.tile([P, 512], f32, tag="ps_proj")
                    for kc in range(KC):
                        nc.tensor.matmul(
                            ps[:],
                            xTh[tci // 4][:, kc, (tci % 4) * P : (tci % 4 + 1) * P],
                            wv_sb[:, kc, :],
                            start=(kc == 0), stop=(kc == KC - 1),
                        )
                    nc.vector.tensor_tensor(
                        v_sb[:, tci, :, 0:64],
                        ps.rearrange("p (h d) -> p h d", d=64),
                        bv_bc.rearrange("p (h d) -> p h d", d=64),
                        mybir.AluOpType.add,
                    )

            for half in range(IC):
                xts = []
                for hp in range(2):
                    for k in range(4):
                        xt = xstage.tile([P, 512], f32, tag="xt", name=f"xt_{half}_{hp}_{k}")
                        nc.sync.dma_start(
                            xt[:],
                            x_d[(half * 4 + k) * P : (half * 4 + k + 1) * P,
                                hp * 512 : (hp + 1) * 512],
                        )
                        xts.append((hp, k, xt))
                if half == 0:
                    nc.sync.dma_start(wv_sb[:], wv_d.bitcast(f32r).rearrange("(kc p) f -> p kc f", p=P))
                xdst = xTh[half]
                for hc in range(KC):
                    hp, hcl = hc // 4, hc % 4
                    pst = ps_q.tile([P, 512], f32, tag="ps_proj")
                    for k in range(4):
                        xt = [t for (h_, k_, t) in xts if h_ == hp and k_ == k][0]
                        nc.tensor.transpose(
                            pst[:, k * P : (k + 1) * P],
                            xt[:, hcl * P : (hcl + 1) * P], ident[:],
                        )
                    nc.vector.tensor_copy(xdst[:, hc, :], pst[:])
                if half == 0:
                    # v-proj of the first token half fills PE while the second
                    # x half is still in flight on the DMA engine
                    emit_vproj(range(0, 4))
            nc.sync.dma_start(wq_sb[:], wq_d.bitcast(f32r).rearrange("(kc p) f -> p kc f", p=P))
            nc.sync.dma_start(wk_sb[:], wk_d.bitcast(f32r).rearrange("(kc p) f -> p kc f", p=P))
            nc.sync.dma_start(cb_sb[:], cb_d.rearrange("(jc p) -> p jc", p=P))
            nc.sync.dma_start(bq_sb[:], bqs_d.rearrange("(fc p) -> p fc", p=P))
            nc.sync.dma_start(bk_sb[:], bk_d.rearrange("(fc p) -> p fc", p=P))
            emit_vproj(range(4, 8))

        # ---- attention interleaved with q projection, per head pair ----
        attn2 = ctx.enter_context(tc.tile_pool(name="attn2", bufs=1))

        # head h -> partitions (h%2)*64.., feature chunk h//2 (pairs stacked on
        # the partition axis so o_proj contracts K=128)
        oT = attn2.tile([P, FC, S], bf16, tag="oT")
        wo_sb = attn2.tile([P, FC, H], bf16, tag="wo_sb")
        nc.gpsimd.dma_start(wo_sb[:], wo_d.rearrange("(fc p) o -> p fc o", p=P))

        def emit_oproj(tci_range):
            for tci in tci_range:
                for oc in range(IC):
                    psz = ps_q.tile([P, 512], f32, tag="ps_proj")
                    for fc in range(FC):
                        nc.tensor.matmul(
                            psz[:],
                            oT[:, fc, tci * P : (tci + 1) * P],
                            wo_sb[:, fc, oc * 512 : (oc + 1) * 512],
                            start=(fc == 0), stop=(fc == FC - 1),
                        )
                    zt = zpool.tile([P, 512], f32, tag="zt")
                    nc.vector.tensor_copy(zt[:], psz[:])
                    nc.sync.dma_start(z_d[tci * P : (tci + 1) * P, oc * 512 : (oc + 1) * 512], zt[:])

        def flush_divisions(pending):
            for (ic_, fc4_, side_, pso_, rlb_) in pending:
                hb = side_ * 64
                nc.vector.tensor_tensor(
                    oT[hb : hb + 64, fc4_, ic_ * 512 : (ic_ + 1) * 512],
                    pso_[0:64, :], rlb_[:],
                    mybir.AluOpType.mult,
                )
            pending.clear()

        pending = []
        ic0_done = False
        for fc4 in range(FC):
            # q then k projection for this pair's feature chunk (DMA arrival order)
            for ic in range(IC):
                ps = ps_q.tile([P, 512], f32, tag="ps_proj")
                for kc in range(KC):
                    nc.tensor.matmul(
                        ps[:],
                        wq_sb[:, kc, fc4 * P : (fc4 + 1) * P],
                        xTh[ic][:, kc, :],
                        start=(kc == 0), stop=(kc == KC - 1),
                    )
                nc.vector.tensor_scalar(
                    qT[:, fc4, ic * 512 : (ic + 1) * 512], ps[:],
                    SCALE, bq_sb[:, fc4 : fc4 + 1],
                    mybir.AluOpType.mult, mybir.AluOpType.add,
                )
            for ic in range(IC):
                ps = ps_q.tile([P, 512], f32, tag="ps_proj")
                for kc in range(KC):
                    nc.tensor.matmul(
                        ps[:],
                        wk_sb[:, kc, fc4 * P : (fc4 + 1) * P],
                        xTh[ic][:, kc, :],
                        start=(kc == 0), stop=(kc == KC - 1),
                    )
                nc.vector.tensor_scalar(
                    kT[:, fc4, ic * 512 : (ic + 1) * 512], ps[:],
                    bk_sb[:, fc4 : fc4 + 1], None, mybir.AluOpType.add,
                )
            for ic in range(IC):
                # heads of the pair interleaved: PE alternates A/B matmuls
                # while ACT/DVE process the other head's exp / verb multiply
                psos = []
                for side in range(2):
                    pso = ps_o.tile([65, 512], f32, tag="pso", name=f"pso_{side}")
                    psos.append(pso)
                for jc in range(TC):
                    pTs = []
                    for side in range(2):
                        hb = side * 64
                        pss = ps_s.tile([P, 512], f32, tag="pss", name=f"pss_{side}")
                        nc.tensor.matmul(
                            pss[:],
                            kT[hb : hb + 64, fc4, jc * P : (jc + 1) * P],
                            qT[hb : hb + 64, fc4, ic * 512 : (ic + 1) * 512],
                            start=True, stop=True,
                        )
                        pT = ppool.tile([P, 512], bf16, tag="pT", name=f"pT_{side}")
                        nc.scalar.activation(
                            pT[:], pss[:], mybir.ActivationFunctionType.Exp,
                            bias=cb_sb[:, jc : jc + 1], scale=1.0,
                        )
                        mul_eng = nc.gpsimd if (jc % 2 == 1 and side == 1) else nc.vector
                        mul_eng.tensor_tensor(
                            pT[:], pT[:], ebT[:, jc, ic * 512 : (ic + 1) * 512],
                            mybir.AluOpType.mult,
                        )
                        pTs.append(pT)
                    for side in range(2):
                        h = 2 * fc4 + side
                        nc.tensor.matmul(
                            psos[side][:],
                            v_sb[:, jc, h, 0:65],
                            pTs[side][:],
                            start=(jc == 0), stop=(jc == TC - 1),
                        )
                # previous group's divisions (their broadcasts completed while
                # this group was streaming) — keeps the DVE stream stall-free
                flush_divisions(pending)
                if fc4 == FC - 1 and ic == 1 and not ic0_done:
                    # all ic=0 oT divided once (fc3, ic0)'s flush ran above
                    emit_oproj(range(0, 4))
                    ic0_done = True
                for side in range(2):
                    pso = psos[side]
                    rl = rlpool.tile([65, 512], f32, tag="rl")
                    nc.vector.reciprocal(rl[64:65, :], pso[64:65, :])
                    rlrow = rlpool.tile([1, 512], f32, tag="rlrow")
                    nc.sync.dma_start(rlrow[:], rl[64:65, :])
                    rlb = rlpool.tile([64, 512], f32, tag="rlb")
                    nc.gpsimd.partition_broadcast(rlb[:], rlrow[:])
                    pending.append((ic, fc4, side, pso, rlb))
        flush_divisions(pending)
        emit_oproj(range(4, 8))

    nc.compile()
    return nc


def _get_compiled():
    global _COMPILED
    if _COMPILED is None:
        _COMPILED = _build()
    return _COMPILED


def _host_morpho(morpho_types):
    """nearest-verb index per (b, i) (-1 if batch has no verb) and col bias."""
    mt = np.asarray(morpho_types)
    pos = np.arange(S)
    dist = np.abs(pos[:, None] - pos[None, :]).astype(np.float32)
    nearest = np.empty((B, S), np.float32)
    for b in range(B):
        is_verb = mt[b] == 2
        if not is_verb.any():
            nearest[b] = -1.0
            continue
        dm = np.where(is_verb[None, :], dist, BIG)
        nearest[b] = np.argmin(dm, axis=-1).astype(np.float32)
    cb = (
        np.float32(ROOT_BIAS * 0.5) * (mt == 0)
        + np.float32(SUFFIX_BIAS * 0.3) * (mt == 1)
    ).astype(np.float32)
    return nearest, cb


def kernel(hidden_states, morpho_types, Wq, bq, Wk, bk, Wv, bv, Wo, bo):
    hidden_states = np.ascontiguousarray(np.asarray(hidden_states, np.float32))
    Wq = np.asarray(Wq, np.float32)
    Wk = np.asarray(Wk, np.float32)
    Wv = np.asarray(Wv, np.float32)
    Wo = np.asarray(Wo, np.float32)
    bq = np.asarray(bq, np.float32)
    bk = np.asarray(bk, np.float32)
    bv = np.asarray(bv, np.float32)
    bo = np.asarray(bo, np.float32)

    nearest, cb = _host_morpho(morpho_types)

    nc = _get_compiled()
    in_maps = []
    for c in range(8):
        b, g = c // G, c % G
        fs = slice(g * F, (g + 1) * F)
        in_maps.append({
            "x": hidden_states[b],
            "wq": np.ascontiguousarray(Wq[:, fs]),
            "wk": np.ascontiguousarray(Wk[:, fs]),
            "wv": np.ascontiguousarray(Wv[:, fs]),
            "wo": np.ascontiguousarray(Wo[fs, :]),
            "bqs": np.ascontiguousarray(bq[fs]) * np.float32(SCALE),
            "bk": np.ascontiguousarray(bk[fs]),
            "bv": np.ascontiguousarray(bv[fs]),
            "nearf": nearest[b],
            "cb": cb[b],
        })

    res = run_bass_kernel_spmd(nc, in_maps, core_ids=list(range(8)))
    out = np.empty((B, S, H), np.float32)
    for b in range(B):
        out[b] = res.results[2 * b]["z"] + res.results[2 * b + 1]["z"] + bo
    return out


# revision 44
# speedup vs baseline: 102.7993x; 1.0486x over previous
"""AgglutinativeAttention Trainium2 kernel.

Full inputs in, full output out. Sharding: 8 cores = (batch b in 0..3) x
(head-group g in 0..1). Each core computes, for its batch b and its 8 heads:
  qT/kT = (x @ W{q,k}[:, gF:(g+1)F])^T   [512 feat, 1024 tok]  (q pre-scaled)
  v     =  x @ Wv[:, gF:(g+1)F]          [1024 tok, 512 feat] (+ones col/head)
  per head: sT = kT_h^T-style scores transposed [j, i] via PE,
  morpho bias injected via identity-matmul (verb one-hot) + per-partition
  activation bias (col bias), pT = exp(sT + bias), oT = v_aug^T @ pT with a
  ones row giving the softmax denominator, divide, then partial
  z = o @ Wo[gF:(g+1)F, :].  Host sums the two per-batch partials + bo.
"""

import numpy as np
from contextlib import ExitStack

import concourse.bass as bass
import concourse.mybir as mybir
import concourse.tile as tile
from concourse import bacc
from concourse.bass_utils import run_bass_kernel_spmd
from concourse.masks import make_identity

B, S, H = 4, 1024, 1024
NH, HD = 16, 64
G = 2                 # head groups (tensor-parallel factor per batch)
F = H // G            # 512 features per core
HPC = NH // G         # 8 heads per core
SCALE = 1.0 / np.sqrt(HD)
VERB_BIAS, ROOT_BIAS, SUFFIX_BIAS = 2.0, 1.5, 1.2
BIG = np.float32(1e9)

f32 = mybir.dt.float32
f32r = mybir.dt.float32r
i32 = mybir.dt.int32

P = 128
KC = H // P           # 8 contraction chunks for projections
TC = S // P           # 8 token chunks of 128
IC = S // 512         # 2 chunks of 512 (matmul free dim)
FC = F // P           # 4 feature chunks per core

_COMPILED = None


def _build():
    nc = bacc.Bacc("TRN2", target_bir_lowering=False, debug=False, num_devices=8)

    x_d = nc.dram_tensor("x", [S, H], f32, kind="ExternalInput").ap()
    wq_d = nc.dram_tensor("wq", [H, F], f32, kind="ExternalInput").ap()
    wk_d = nc.dram_tensor("wk", [H, F], f32, kind="ExternalInput").ap()
    wv_d = nc.dram_tensor("wv", [H, F], f32, kind="ExternalInput").ap()
    wo_d = nc.dram_tensor("wo", [F, H], f32, kind="ExternalInput").ap()
    bqs_d = nc.dram_tensor("bqs", [F], f32, kind="ExternalInput").ap()
    bk_d = nc.dram_tensor("bk", [F], f32, kind="ExternalInput").ap()
    bv_d = nc.dram_tensor("bv", [F], f32, kind="ExternalInput").ap()
    nearf_d = nc.dram_tensor("nearf", [S], f32, kind="ExternalInput").ap()
    cb_d = nc.dram_tensor("cb", [S], f32, kind="ExternalInput").ap()
    z_d = nc.dram_tensor("z", [S, H], f32, kind="ExternalOutput").ap()

    with tile.TileContext(nc) as tc, ExitStack() as ctx:
        const = ctx.enter_context(tc.tile_pool(name="const", bufs=1))
        big = ctx.enter_context(tc.tile_pool(name="big", bufs=1))
        ppool = ctx.enter_context(tc.tile_pool(name="ppool", bufs=4))
        rlpool = ctx.enter_context(tc.tile_pool(name="rlpool", bufs=4))
        osbpool = ctx.enter_context(tc.tile_pool(name="osbpool", bufs=4))
        zpool = ctx.enter_context(tc.tile_pool(name="zpool", bufs=2))
        ps_q = ctx.enter_context(tc.tile_pool(name="ps_q", bufs=2, space="PSUM"))
        ps_s = ctx.enter_context(tc.tile_pool(name="ps_s", bufs=2, space="PSUM"))
        ps_o = ctx.enter_context(tc.tile_pool(name="ps_o", bufs=2, space="PSUM"))

        # ---- constants ----
        ident = const.tile([P, P], f32, tag="ident")
        make_identity(nc, ident)
        iota_i = const.tile([P, KC], i32, tag="iota_i")
        nc.gpsimd.iota(iota_i[:], pattern=[[P, KC]], base=0, channel_multiplier=1)
        iota_f = const.tile([P, KC], f32, tag="iota_f")
        nc.vector.tensor_copy(iota_f[:], iota_i[:])

        near_row = const.tile([1, S], f32, tag="near_row")
        nc.sync.dma_start(near_row[:], nearf_d[None, :])
        near_bc = const.tile([P, S], f32, tag="near_bc")
        nc.gpsimd.partition_broadcast(near_bc[:], near_row[:])
        bv_row = const.tile([1, F], f32, tag="bv_row")
        nc.sync.dma_start(bv_row[:], bv_d[None, :])
        bv_bc = const.tile([P, F], f32, tag="bv_bc")
        nc.gpsimd.partition_broadcast(bv_bc[:], bv_row[:])
        cb_sb = const.tile([P, TC], f32, tag="cb_sb")
        bq_sb = const.tile([P, FC], f32, tag="bq_sb")
        bk_sb = const.tile([P, FC], f32, tag="bk_sb")

        bf16 = mybir.dt.bfloat16
        qT = big.tile([P, FC, S], f32r, tag="qT")
        kT = big.tile([P, FC, S], f32r, tag="kT")
        v_sb = big.tile([P, TC, HPC, 65], bf16, tag="v_sb")
        ones64 = const.tile([P, TC * HPC], f32, tag="ones64")
        nc.vector.memset(ones64[:], 1.0)
        nc.vector.tensor_copy(
            v_sb[:, :, :, 64:65],
            ones64.rearrange("p (a b one) -> p a b one", a=TC, b=HPC, one=1),
        )

        # verb factor (transposed): ebT[p, jc, i] = exp(2 * (jc*128+p == nearest[i]))
        # computed up front so its ACT work doesn't sit between the k-proj and
        # q-proj copies in the (in-order) ACT stream
        ebT = big.tile([P, TC, S], bf16, tag="ebT")
        ohstage = ctx.enter_context(tc.tile_pool(name="ohstage", bufs=1))
        for jc in range(TC):
            ohst = ohstage.tile([P, S], f32, tag="ohst")
            nc.vector.tensor_scalar(
                ohst[:], near_bc[:], iota_f[:, jc : jc + 1], 2.0,
                mybir.AluOpType.is_equal, mybir.AluOpType.mult,
            )
            nc.scalar.activation(
                ebT[:, jc, :], ohst[:], mybir.ActivationFunctionType.Exp
            )

        projpool = ctx.enter_context(tc.tile_pool(name="projpool", bufs=1))
        wq_sb = projpool.tile([P, KC, F], f32r, tag="wq_sb")
        wk_sb = projpool.tile([P, KC, F], f32r, tag="wk_sb")
        xTh = []
        for i in range(IC):
            xthalf = projpool.tile([P, KC, 512], f32r, tag=f"xT{i}", name=f"xT{i}")
            xTh.append(xthalf)

        # ---- transposes + v/k projections (wk/wv freed after) ----
        with tc.tile_pool(name="wkvpool", bufs=1) as wkvpool, \
             tc.tile_pool(name="xstage", bufs=8) as xstage:
            wv_sb = wkvpool.tile([P, KC, F], f32r, tag="wv_sb")
            # x DMAs + transposes first; weights behind them on the DMA engine
            def emit_vproj(tci_range):
                for tci in tci_range:
                    ps = ps_q.tile([P, 512], f32, tag="ps_proj")
                    for kc in range(KC):
                        nc.tensor.matmul(
                            ps[:],
                            xTh[tci // 4][:, kc, (tci % 4) * P : (tci % 4 + 1) * P],
                            wv_sb[:, kc, :],
                            start=(kc == 0), stop=(kc == KC - 1),
                        )
                    nc.vector.tensor_tensor(
                        v_sb[:, tci, :, 0:64],
                        ps.rearrange("p (h d) -> p h d", d=64),
                        bv_bc.rearrange("p (h d) -> p h d", d=64),
                        mybir.AluOpType.add,
                    )

            for half in range(IC):
                xts = []
                for hp in range(2):
                    for k in range(4):
                        xt = xstage.tile([P, 512], f32, tag="xt", name=f"xt_{half}_{hp}_{k}")
                        nc.sync.dma_start(
                            xt[:],
                            x_d[(half * 4 + k) * P : (half * 4 + k + 1) * P,
                                hp * 512 : (hp + 1) * 512],
                        )
                        xts.append((hp, k, xt))
                if half == 0:
                    nc.sync.dma_start(wv_sb[:], wv_d.bitcast(f32r).rearrange("(kc p) f -> p kc f", p=P))
                xdst = xTh[half]
                for hc in range(KC):
                    hp, hcl = hc // 4, hc % 4
                    pst = ps_q.tile([P, 512], f32, tag="ps_proj")
                    for k in range(4):
                        xt = [t for (h_, k_, t) in xts if h_ == hp and k_ == k][0]
                        nc.tensor.transpose(
                            pst[:, k * P : (k + 1) * P],
                            xt[:, hcl * P : (hcl + 1) * P], ident[:],
                        )
                    if hc % 2 == 0:
                        nc.vector.tensor_copy(xdst[:, hc, :], pst[:])
                    else:
                        nc.scalar.copy(xdst[:, hc, :], pst[:])
                if half == 0:
                    # v-proj of the first token half fills PE while the second
                    # x half is still in flight on the DMA engine
                    emit_vproj(range(0, 4))
            nc.sync.dma_start(wq_sb[:], wq_d.bitcast(f32r).rearrange("(kc p) f -> p kc f", p=P))
            nc.sync.dma_start(wk_sb[:], wk_d.bitcast(f32r).rearrange("(kc p) f -> p kc f", p=P))
            nc.sync.dma_start(cb_sb[:], cb_d.rearrange("(jc p) -> p jc", p=P))
            nc.sync.dma_start(bq_sb[:], bqs_d.rearrange("(fc p) -> p fc", p=P))
            nc.sync.dma_start(bk_sb[:], bk_d.rearrange("(fc p) -> p fc", p=P))
            emit_vproj(range(4, 8))

        # ---- attention interleaved with q projection, per head pair ----
        attn2 = ctx.enter_context(tc.tile_pool(name="attn2", bufs=1))

        # head h -> partitions (h%2)*64.., feature chunk h//2 (pairs stacked on
        # the partition axis so o_proj contracts K=128)
        oT = attn2.tile([P, FC, S], bf16, tag="oT")
        wo_sb = attn2.tile([P, FC, H], bf16, tag="wo_sb")
        nc.gpsimd.dma_start(wo_sb[:], wo_d.rearrange("(fc p) o -> p fc o", p=P))

        def emit_oproj(tci_range, tail=False):
            for tci in tci_range:
                for oc in range(IC):
                    psz = ps_q.tile([P, 512], f32, tag="ps_proj")
                    for fc in range(FC):
                        nc.tensor.matmul(
                            psz[:],
                            oT[:, fc, tci * P : (tci + 1) * P],
                            wo_sb[:, fc, oc * 512 : (oc + 1) * 512],
                            start=(fc == 0), stop=(fc == FC - 1),
                        )
                    zt = zpool.tile([P, 512], f32, tag="zt")
                    if tail:
                        nc.scalar.copy(zt[:], psz[:])
                    else:
                        nc.vector.tensor_copy(zt[:], psz[:])
                    nc.sync.dma_start(z_d[tci * P : (tci + 1) * P, oc * 512 : (oc + 1) * 512], zt[:])

        def flush_divisions(pending, eng=None):
            for (ic_, fc4_, side_, osb_, rlb_) in pending:
                hb = side_ * 64
                (eng or nc.gpsimd).tensor_tensor(
                    oT[hb : hb + 64, fc4_, ic_ * 512 : (ic_ + 1) * 512],
                    osb_[0:64, :], rlb_[:],
                    mybir.AluOpType.mult,
                )
            pending.clear()

        def emit_qproj(fc, icq):
            ps = ps_q.tile([P, 512], f32, tag="ps_proj")
            for kc in range(KC):
                nc.tensor.matmul(
                    ps[:],
                    wq_sb[:, kc, fc * P : (fc + 1) * P],
                    xTh[icq][:, kc, :],
                    start=(kc == 0), stop=(kc == KC - 1),
                )
            nc.vector.tensor_scalar(
                qT[:, fc, icq * 512 : (icq + 1) * 512], ps[:],
                SCALE, bq_sb[:, fc : fc + 1],
                mybir.AluOpType.mult, mybir.AluOpType.add,
            )

        def emit_kproj(fc, ick):
            ps = ps_q.tile([P, 512], f32, tag="ps_proj")
            for kc in range(KC):
                nc.tensor.matmul(
                    ps[:],
                    wk_sb[:, kc, fc * P : (fc + 1) * P],
                    xTh[ick][:, kc, :],
                    start=(kc == 0), stop=(kc == KC - 1),
                )
            nc.vector.tensor_scalar(
                kT[:, fc, ick * 512 : (ick + 1) * 512], ps[:],
                bk_sb[:, fc : fc + 1], None, mybir.AluOpType.add,
            )

        pending = []
        ic0_done = False
        for fc4 in range(FC):
            # only the first-half projections gate the pair start; second
            # halves ride inside the first group's early steps
            emit_qproj(fc4, 0)
            emit_kproj(fc4, 0)
            for ic in range(IC):
                # heads of the pair interleaved: PE alternates A/B matmuls
                # while ACT/DVE process the other head's exp / verb multiply
                psos = []
                for side in range(2):
                    pso = ps_o.tile([65, 512], f32, tag="pso", name=f"pso_{side}")
                    psos.append(pso)
                for jc in range(TC):
                    # both heads' scores into one 2-bank psum tile so a single
                    # [128,1024] exp (and verb multiply) covers the pair —
                    # halves the instruction count and PSUM-access overhead on
                    # the ACT-critical path
                    pssb = ps_s.tile([P, 1024], f32, tag="pssb")
                    for side in range(2):
                        hb = side * 64
                        nc.tensor.matmul(
                            pssb[:, side * 512 : (side + 1) * 512],
                            kT[hb : hb + 64, fc4, jc * P : (jc + 1) * P],
                            qT[hb : hb + 64, fc4, ic * 512 : (ic + 1) * 512],
                            start=True, stop=True,
                        )
                    pTb = ppool.tile([P, 1024], bf16, tag="pTb")
                    nc.scalar.activation(
                        pTb[:], pssb[:], mybir.ActivationFunctionType.Exp,
                        bias=cb_sb[:, jc : jc + 1], scale=1.0,
                    )
                    ebsl = ebT[:, jc, ic * 512 : (ic + 1) * 512]
                    nc.vector.tensor_tensor(
                        pTb.rearrange("p (two n) -> p two n", two=2),
                        pTb.rearrange("p (two n) -> p two n", two=2),
                        ebsl[:, None, :].to_broadcast((P, 2, 512)),
                        mybir.AluOpType.mult,
                    )
                    if ic == 0 and jc == 0:
                        emit_kproj(fc4, 1)
                    if ic == 0 and jc == 2:
                        emit_qproj(fc4, 1)
                    for side in range(2):
                        h = 2 * fc4 + side
                        nc.tensor.matmul(
                            psos[side][:],
                            v_sb[:, jc, h, 0:65],
                            pTb[:, side * 512 : (side + 1) * 512],
                            start=(jc == 0), stop=(jc == TC - 1),
                        )
                # previous group's divisions (their broadcasts completed while
                # this group was streaming) — keeps the DVE stream stall-free
                flush_divisions(pending)
                if fc4 == FC - 1 and ic == 1 and not ic0_done:
                    # all ic=0 oT divided once (fc3, ic0)'s flush ran above
                    emit_oproj(range(0, 4))
                    ic0_done = True
                for side in range(2):
                    pso = psos[side]
                    osb = osbpool.tile([65, 512], f32, tag="osb", name=f"osb_{side}")
                    nc.vector.tensor_copy(osb[:], pso[:])
                    rlrow = rlpool.tile([1, 512], f32, tag="rlrow")
                    nc.vector.reciprocal(rlrow[:], osb[64:65, :])
                    rlb = rlpool.tile([64, 512], f32, tag="rlb")
                    nc.gpsimd.partition_broadcast(rlb[:], rlrow[:])
                    pending.append((ic, fc4, side, osb, rlb))
        flush_divisions(pending, eng=nc.vector)
        emit_oproj(range(4, 8), tail=True)

    nc.compile()
    return nc


def _get_compiled():
    global _COMPILED
    if _COMPILED is None:
        _COMPILED = _build()
    return _COMPILED


def _host_morpho(morpho_types):
    """nearest-verb index per (b, i) (-1 if batch has no verb) and col bias."""
    mt = np.asarray(morpho_types)
    pos = np.arange(S)
    dist = np.abs(pos[:, None] - pos[None, :]).astype(np.float32)
    nearest = np.empty((B, S), np.float32)
    for b in range(B):
        is_verb = mt[b] == 2
        if not is_verb.any():
            nearest[b] = -1.0
            continue
        dm = np.where(is_verb[None, :], dist, BIG)
        nearest[b] = np.argmin(dm, axis=-1).astype(np.float32)
    cb = (
        np.float32(ROOT_BIAS * 0.5) * (mt == 0)
        + np.float32(SUFFIX_BIAS * 0.3) * (mt == 1)
    ).astype(np.float32)
    return nearest, cb


def kernel(hidden_states, morpho_types, Wq, bq, Wk, bk, Wv, bv, Wo, bo):
    hidden_states = np.ascontiguousarray(np.asarray(hidden_states, np.float32))
    Wq = np.asarray(Wq, np.float32)
    Wk = np.asarray(Wk, np.float32)
    Wv = np.asarray(Wv, np.float32)
    Wo = np.asarray(Wo, np.float32)
    bq = np.asarray(bq, np.float32)
    bk = np.asarray(bk, np.float32)
    bv = np.asarray(bv, np.float32)
    bo = np.asarray(bo, np.float32)

    nearest, cb = _host_morpho(morpho_types)

    nc = _get_compiled()
    in_maps = []
    for c in range(8):
        b, g = c // G, c % G
        fs = slice(g * F, (g + 1) * F)
        in_maps.append({
            "x": hidden_states[b],
            "wq": np.ascontiguousarray(Wq[:, fs]),
            "wk": np.ascontiguousarray(Wk[:, fs]),
            "wv": np.ascontiguousarray(Wv[:, fs]),
            "wo": np.ascontiguousarray(Wo[fs, :]),
            "bqs": np.ascontiguousarray(bq[fs]) * np.float32(SCALE),
            "bk": np.ascontiguousarray(bk[fs]),
            "bv": np.ascontiguousarray(bv[fs]),
            "nearf": nearest[b],
            "cb": cb[b],
        })

    res = run_bass_kernel_spmd(nc, in_maps, core_ids=list(range(8)))
    out = np.empty((B, S, H), np.float32)
    for b in range(B):
        out[b] = res.results[2 * b]["z"] + res.results[2 * b + 1]["z"] + bo
    return out
